# revision 10
# baseline (speedup 1.0000x reference)
"""Trainium2 Bass kernel for nn_LtiRnn: batched nonlinear RNN scan.

Math (per sample, row-vector form, T=2048 steps):
    z_t = (x_t @ C2.T + u_t @ D21.T) * t_inv
    w_t = tanh(z_t)
    y_t = x_t @ C1.T + u_t @ D11.T + w_t @ D12.T
    x_{t+1} = (x_t @ A.T + u_t @ B1.T + w_t @ B2.T) @ Y_inv

Device formulation (column-major / feature-on-partitions, per core batch Bc=32):
    V_s = [eps * X_s ; W_s]  in SBUF, [128, 32]  (X [64,Bc], W [64,Bc])
    One PE matmul per step computes P_{s+1} = [eps*X_{s+1} ; Z_{s+1}] into PSUM
    (u-dependent bias terms pre-accumulated into PSUM by batched group matmuls),
    then ONE ScalarE tanh over all 128 rows produces V_{s+1}:
      - rows 64:128: tanh(Z) = W  (the real nonlinearity)
      - rows 0:64: tanh(eps*X) == eps*X to ~1e-10 rel (eps=2^-16), so the same
        instruction "copies" the linear state, avoiding a second engine hop.
    y_t computed off the critical path in 16-step groups from the stored V history.

Sharding: data-parallel over batch across 8 cores (32 samples each), weights
replicated. SPMD single NEFF via run_bass_kernel_spmd / PJRT under axon.

NOTE: all matmuls keep lhsT/rhs at base partition 0 — K=32 matmuls with
tile_position row offsets (32/64/96) crash the exec unit on this stack.
"""

import numpy as np

NX, NU, NY, NW = 64, 32, 32, 64
B, T = 256, 2048
NCORES = 8
BC = B // NCORES  # 32 samples per core
EPS = 2.0 ** -16
G = 16            # steps per group
CH = 256          # steps per SBUF input chunk

_cached = {}


def _build_bass():
    import concourse.tile as tile
    from concourse import bacc, mybir

    s_tot = T + 1                      # states V_0 .. V_T
    ngroups = (s_tot + G - 1) // G     # last group holds only s=T
    nch = T // CH

    f32 = mybir.dt.float32
    nc = bacc.Bacc(trn_type="TRN2")

    u_d = nc.dram_tensor("u_feat", [NU, T * BC], f32, kind="ExternalInput")
    wstep_d = nc.dram_tensor("w_step", [128, 128], f32, kind="ExternalInput")
    wu1_d = nc.dram_tensor("wu1", [NU, 128], f32, kind="ExternalInput")
    wu2_d = nc.dram_tensor("wu2", [NU, 128], f32, kind="ExternalInput")
    wy_d = nc.dram_tensor("w_y", [128, NY], f32, kind="ExternalInput")
    wyu_d = nc.dram_tensor("w_yu", [NU, NY], f32, kind="ExternalInput")
    y_d = nc.dram_tensor("y_out", [NY, T * BC], f32, kind="ExternalOutput")
    xf_d = nc.dram_tensor("x_final", [NX, BC], f32, kind="ExternalOutput")

    with tile.TileContext(nc) as tc:
        with (
            tc.tile_pool(name="singles", bufs=1) as singles,
            tc.tile_pool(name="upool", bufs=4) as upool,
            tc.tile_pool(name="vpool", bufs=4) as vpool,
            tc.tile_pool(name="ystage", bufs=4) as ystage,
            tc.tile_pool(name="ppool", bufs=4, space="PSUM") as ppool,
            tc.tile_pool(name="ypsum", bufs=2, space="PSUM") as ypsum,
        ):
            wstep_t = singles.tile([128, 128], f32)
            nc.gpsimd.dma_start(out=wstep_t, in_=wstep_d[:, :])
            wu1_t = singles.tile([NU, 128], f32)
            nc.gpsimd.dma_start(out=wu1_t, in_=wu1_d[:, :])
            wu2_t = singles.tile([NU, 128], f32)
            nc.gpsimd.dma_start(out=wu2_t, in_=wu2_d[:, :])
            wy_t = singles.tile([128, NY], f32)
            nc.gpsimd.dma_start(out=wy_t, in_=wy_d[:, :])
            wyu_t = singles.tile([NU, NY], f32)
            nc.gpsimd.dma_start(out=wyu_t, in_=wyu_d[:, :])

            chunk_tiles = {}

            def ensure_chunk(c):
                if c < 0 or c >= nch or c in chunk_tiles:
                    return
                ut = upool.tile([NU, CH * BC], f32, name=f"u_{c}", tag="u")
                nc.gpsimd.dma_start(
                    out=ut, in_=u_d[:, c * CH * BC:(c + 1) * CH * BC])
                chunk_tiles[c] = ut

            ensure_chunk(0)
            ensure_chunk(1)

            def u_ap(s0, n):
                """AP over input chunk for n consecutive steps starting at s0
                (must stay within one chunk)."""
                c = s0 // CH
                col = (s0 % CH) * BC
                return chunk_tiles[c][:, col:col + n * BC]

            def emit_bias(p_t, slot0, s0, n, w_t, start):
                """p_t[:, slot0:slot0+n, :] (+)= w.T @ [U_{s0} .. U_{s0+n-1}],
                splitting runs at chunk boundaries."""
                done = 0
                while done < n:
                    s = s0 + done
                    run = min(n - done, CH - (s % CH))
                    nc.tensor.matmul(
                        p_t[:, slot0 + done:slot0 + done + run, :],
                        w_t,
                        u_ap(s, run),
                        start=start and done == 0,
                        stop=False,
                        skip_group_check=True,
                    )
                    done += run

            v_tiles = {}
            # deferred off-critical-path work (y matmuls), interleaved between
            # critical-path steps so the PE executes them inside tanh windows
            # instead of in one burst at group boundaries.
            pending = []

            for g in range(ngroups):
                s_lo = g * G
                s_hi = min(s_lo + G, s_tot)
                n_here = s_hi - s_lo

                p_t = ppool.tile([128, G, BC], f32, name=f"p_{g}", tag="p")
                v_t = vpool.tile([128, G, BC], f32, name=f"v_{g}", tag="v")
                v_tiles[g] = v_t

                # u-dependent bias terms for this group, into PSUM
                if g == 0:
                    # slot i <- wu2.T @ U_i   (i = 0..15), first writer
                    emit_bias(p_t, 0, 0, G, wu2_t, start=True)
                    # slot i <- wu1.T @ U_{i-1}  (i = 1..15)
                    emit_bias(p_t, 1, 0, G - 1, wu1_t, start=False)
                elif g < ngroups - 1:
                    emit_bias(p_t, 0, s_lo - 1, G, wu1_t, start=True)
                    emit_bias(p_t, 0, s_lo, G, wu2_t, start=False)
                else:
                    emit_bias(p_t, 0, s_lo - 1, n_here, wu1_t, start=True)

                # prefetch the chunk two windows out once this group's bias
                # (which may read the previous chunk's tail) has been emitted
                if s_lo % CH == 0:
                    ensure_chunk(s_lo // CH + 2)

                for i in range(n_here):
                    s = s_lo + i
                    if s >= 1:
                        pg, pi = (s - 1) // G, (s - 1) % G
                        nc.tensor.matmul(
                            p_t[:, i, :],
                            wstep_t,
                            v_tiles[pg][:, pi, :],
                            start=False,
                            stop=True,
                            skip_group_check=True,
                        )
                    nc.scalar.activation(
                        v_t[:, i, :],
                        p_t[:, i, :],
                        mybir.ActivationFunctionType.Tanh,
                    )
                    # interleave one piece of deferred work per step
                    if pending:
                        pending.pop(0)()

                if g >= 4 and g - 4 in v_tiles:
                    del v_tiles[g - 4]

                # y for steps [16g, 16g+16) — deferred into the next group's
                # step loop
                if g < ngroups - 1:
                    def make_y(g=g, v_t=v_t, s_lo=s_lo):
                        def do_y1():
                            yp = ypsum.tile(
                                [NY, G * BC], f32, name=f"yp_{g}", tag="yp")
                            nc.tensor.matmul(
                                yp, wy_t, v_t[:, :, :],
                                start=True, stop=False, skip_group_check=True,
                            )
                            def do_y2():
                                nc.tensor.matmul(
                                    yp, wyu_t, u_ap(s_lo, G),
                                    start=False, stop=True,
                                    skip_group_check=True,
                                )
                                def do_y3():
                                    ys = ystage.tile(
                                        [NY, G * BC], f32,
                                        name=f"ys_{g}", tag="ys")
                                    nc.vector.tensor_copy(ys, yp)
                                    nc.gpsimd.dma_start(
                                        out=y_d[:, s_lo * BC:(s_lo + G) * BC],
                                        in_=ys)
                                pending.append(do_y3)
                            pending.append(do_y2)
                        return do_y1
                    pending.append(make_y())

            while pending:
                pending.pop(0)()

            # x_final = V_T rows 0:64 (still eps-scaled; host rescales)
            nc.gpsimd.dma_start(
                out=xf_d[:, :], in_=v_tiles[ngroups - 1][0:NX, 0, :]
            )

    nc.finalize()
    return nc


def _host_prep(x_pred, Y, lambdas, A, B1, B2, C1, D11, D12, C2, D21):
    """Precompute folded weights + per-core packed inputs (all float32)."""
    f = np.float32
    Y_inv = np.linalg.inv(Y.astype(np.float64))
    t_inv = 1.0 / lambdas.astype(np.float64)[:, 0]

    Ab = Y_inv.T @ A.astype(np.float64)   # [nx, nx]
    B1b = Y_inv.T @ B1.astype(np.float64)  # [nx, nu]
    B2b = Y_inv.T @ B2.astype(np.float64)  # [nx, nw]
    C2b = t_inv[:, None] * C2.astype(np.float64)  # [nw, nx]
    D21b = t_inv[:, None] * D21.astype(np.float64)  # [nw, nu]
    C2A = C2b @ Ab    # [nw, nx]
    C2B2 = C2b @ B2b  # [nw, nw]
    C2B1 = C2b @ B1b  # [nw, nu]

    w_step = np.zeros((128, 128), f)
    w_step[0:NX, 0:NX] = Ab.T
    w_step[NX:128, 0:NX] = EPS * B2b.T
    w_step[0:NX, NX:128] = (1.0 / EPS) * C2A.T
    w_step[NX:128, NX:128] = C2B2.T

    wu1 = np.zeros((NU, 128), f)
    wu1[:, 0:NX] = EPS * B1b.T
    wu1[:, NX:128] = C2B1.T
    wu2 = np.zeros((NU, 128), f)
    wu2[:, NX:128] = D21b.T

    w_y = np.zeros((128, NY), f)
    w_y[0:NX, :] = (1.0 / EPS) * C1.T
    w_y[NX:128, :] = D12.T
    w_yu = D11.T.astype(f).copy()

    # feature-major U per core: [nu, T*Bc], col = t*Bc + b
    u_cores = []
    for c in range(NCORES):
        xs = x_pred[c * BC:(c + 1) * BC, :T].astype(f)  # [Bc, T, nu]
        u_cores.append(
            np.ascontiguousarray(xs.transpose(2, 1, 0).reshape(NU, T * BC)))
    return u_cores, dict(w_step=w_step, wu1=wu1, wu2=wu2, w_y=w_y, w_yu=w_yu)


def kernel(x_pred, Y, lambdas, A, B1, B2, C1, D11, D12, C2, D21):
    from concourse.bass_utils import run_bass_kernel_spmd

    u_cores, weights = _host_prep(
        x_pred, Y, lambdas, A, B1, B2, C1, D11, D12, C2, D21)

    if "nc" not in _cached:
        _cached["nc"] = _build_bass()
    nc = _cached["nc"]

    in_maps = [dict(u_feat=u_cores[c], **weights) for c in range(NCORES)]
    res = run_bass_kernel_spmd(nc, in_maps, core_ids=list(range(NCORES)))

    y = np.empty((B, T, NY), np.float32)
    x_final = np.empty((B, NX), np.float32)
    for c in range(NCORES):
        yc = res.results[c]["y_out"].reshape(NY, T, BC)
        y[c * BC:(c + 1) * BC] = yc.transpose(2, 1, 0)
        x_final[c * BC:(c + 1) * BC] = res.results[c]["x_final"].T * (1.0 / EPS)
    return y, x_final


# revision 11
# speedup vs baseline: 1.1856x; 1.1856x over previous
"""Trainium2 Bass kernel for nn_LtiRnn: batched nonlinear RNN scan.

Math (per sample, row-vector form, T=2048 steps):
    z_t = (x_t @ C2.T + u_t @ D21.T) * t_inv
    w_t = tanh(z_t)
    y_t = x_t @ C1.T + u_t @ D11.T + w_t @ D12.T
    x_{t+1} = (x_t @ A.T + u_t @ B1.T + w_t @ B2.T) @ Y_inv

Device formulation (column-major / feature-on-partitions, per core batch Bc=32):
    V_s = [eps * X_s ; W_s]  in SBUF, [128, 32]  (X [64,Bc], W [64,Bc])
    One PE matmul per step computes P_{s+1} = [eps*X_{s+1} ; Z_{s+1}] into PSUM
    (u-dependent bias terms pre-accumulated into PSUM by batched group matmuls),
    then ONE ScalarE tanh over all 128 rows produces V_{s+1}:
      - rows 64:128: tanh(Z) = W  (the real nonlinearity)
      - rows 0:64: tanh(eps*X) == eps*X to ~1e-10 rel (eps=2^-16), so the same
        instruction "copies" the linear state, avoiding a second engine hop.
    y_t computed off the critical path in 16-step groups from the stored V history.

Sharding: data-parallel over batch across 8 cores (32 samples each), weights
replicated. SPMD single NEFF via run_bass_kernel_spmd / PJRT under axon.

NOTE: all matmuls keep lhsT/rhs at base partition 0 — K=32 matmuls with
tile_position row offsets (32/64/96) crash the exec unit on this stack.
"""

import os
import numpy as np

VARIANT = os.environ.get("KERNEL_VARIANT", "full")  # full | noy | chainonly

NX, NU, NY, NW = 64, 32, 32, 64
B, T = 256, 2048
NCORES = 8
BC = B // NCORES  # 32 samples per core
EPS = 2.0 ** -16
G = 16            # steps per group
CH = 256          # steps per SBUF input chunk

_cached = {}


def _build_bass():
    import concourse.tile as tile
    from concourse import bacc, mybir

    s_tot = T + 1                      # states V_0 .. V_T
    ngroups = (s_tot + G - 1) // G     # last group holds only s=T
    nch = T // CH

    f32 = mybir.dt.float32
    nc = bacc.Bacc(trn_type="TRN2")

    u_d = nc.dram_tensor("u_feat", [NU, T * BC], f32, kind="ExternalInput")
    wstep_d = nc.dram_tensor("w_step", [128, 128], f32, kind="ExternalInput")
    wu1_d = nc.dram_tensor("wu1", [NU, 128], f32, kind="ExternalInput")
    wu2_d = nc.dram_tensor("wu2", [NU, 128], f32, kind="ExternalInput")
    wy_d = nc.dram_tensor("w_y", [128, NY], f32, kind="ExternalInput")
    wyu_d = nc.dram_tensor("w_yu", [NU, NY], f32, kind="ExternalInput")
    y_d = nc.dram_tensor("y_out", [NY, T * BC], f32, kind="ExternalOutput")
    xf_d = nc.dram_tensor("x_final", [NX, BC], f32, kind="ExternalOutput")

    with tile.TileContext(nc) as tc:
        with (
            tc.tile_pool(name="singles", bufs=1) as singles,
            tc.tile_pool(name="upool", bufs=4) as upool,
            tc.tile_pool(name="vpool", bufs=4) as vpool,
            tc.tile_pool(name="ystage", bufs=4) as ystage,
            tc.tile_pool(name="ppool", bufs=4, space="PSUM") as ppool,
            tc.tile_pool(name="ypsum", bufs=2, space="PSUM") as ypsum,
        ):
            wstep_t = singles.tile([128, 128], f32)
            nc.gpsimd.dma_start(out=wstep_t, in_=wstep_d[:, :])
            wu1_t = singles.tile([NU, 128], f32)
            nc.gpsimd.dma_start(out=wu1_t, in_=wu1_d[:, :])
            wu2_t = singles.tile([NU, 128], f32)
            nc.gpsimd.dma_start(out=wu2_t, in_=wu2_d[:, :])
            wy_t = singles.tile([128, NY], f32)
            nc.gpsimd.dma_start(out=wy_t, in_=wy_d[:, :])
            wyu_t = singles.tile([NU, NY], f32)
            nc.gpsimd.dma_start(out=wyu_t, in_=wyu_d[:, :])

            chunk_tiles = {}

            def ensure_chunk(c):
                if c < 0 or c >= nch or c in chunk_tiles:
                    return
                ut = upool.tile([NU, CH * BC], f32, name=f"u_{c}", tag="u")
                nc.gpsimd.dma_start(
                    out=ut, in_=u_d[:, c * CH * BC:(c + 1) * CH * BC])
                chunk_tiles[c] = ut

            ensure_chunk(0)
            ensure_chunk(1)

            def u_ap(s0, n):
                """AP over input chunk for n consecutive steps starting at s0
                (must stay within one chunk)."""
                c = s0 // CH
                col = (s0 % CH) * BC
                return chunk_tiles[c][:, col:col + n * BC]

            def emit_bias(p_t, slot0, s0, n, w_t, start):
                """p_t[:, slot0:slot0+n, :] (+)= w.T @ [U_{s0} .. U_{s0+n-1}],
                splitting runs at chunk boundaries."""
                done = 0
                while done < n:
                    s = s0 + done
                    run = min(n - done, CH - (s % CH))
                    nc.tensor.matmul(
                        p_t[:, slot0 + done:slot0 + done + run, :],
                        w_t,
                        u_ap(s, run),
                        start=start and done == 0,
                        stop=False,
                        skip_group_check=True,
                    )
                    done += run

            v_tiles = {}
            # deferred off-critical-path work (y matmuls), interleaved between
            # critical-path steps so the PE executes them inside tanh windows
            # instead of in one burst at group boundaries.
            pending = []

            for g in range(ngroups):
                s_lo = g * G
                s_hi = min(s_lo + G, s_tot)
                n_here = s_hi - s_lo

                p_t = ppool.tile([128, G, BC], f32, name=f"p_{g}", tag="p")
                v_t = vpool.tile([128, G, BC], f32, name=f"v_{g}", tag="v")
                v_tiles[g] = v_t

                # u-dependent bias terms for this group, into PSUM
                if VARIANT == "chainonly":
                    pass
                elif g == 0:
                    # slot i <- wu2.T @ U_i   (i = 0..15), first writer
                    emit_bias(p_t, 0, 0, G, wu2_t, start=True)
                    # slot i <- wu1.T @ U_{i-1}  (i = 1..15)
                    emit_bias(p_t, 1, 0, G - 1, wu1_t, start=False)
                elif g < ngroups - 1:
                    emit_bias(p_t, 0, s_lo - 1, G, wu1_t, start=True)
                    emit_bias(p_t, 0, s_lo, G, wu2_t, start=False)
                else:
                    emit_bias(p_t, 0, s_lo - 1, n_here, wu1_t, start=True)

                # prefetch the chunk two windows out once this group's bias
                # (which may read the previous chunk's tail) has been emitted
                if s_lo % CH == 0:
                    ensure_chunk(s_lo // CH + 2)

                for i in range(n_here):
                    s = s_lo + i
                    if s >= 1:
                        pg, pi = (s - 1) // G, (s - 1) % G
                        nc.tensor.matmul(
                            p_t[:, i, :],
                            wstep_t,
                            v_tiles[pg][:, pi, :],
                            start=(VARIANT == "chainonly"),
                            stop=True,
                            skip_group_check=True,
                        )
                    nc.scalar.activation(
                        v_t[:, i, :],
                        p_t[:, i, :],
                        mybir.ActivationFunctionType.Tanh,
                    )
                    # interleave one piece of deferred work per step
                    if pending:
                        pending.pop(0)()

                if g >= 4 and g - 4 in v_tiles:
                    del v_tiles[g - 4]

                # y for steps [16g, 16g+16) — deferred into the next group's
                # step loop
                if g < ngroups - 1 and VARIANT == "full":
                    def make_y(g=g, v_t=v_t, s_lo=s_lo):
                        def do_y1():
                            yp = ypsum.tile(
                                [NY, G * BC], f32, name=f"yp_{g}", tag="yp")
                            nc.tensor.matmul(
                                yp, wy_t, v_t[:, :, :],
                                start=True, stop=False, skip_group_check=True,
                            )
                            def do_y2():
                                nc.tensor.matmul(
                                    yp, wyu_t, u_ap(s_lo, G),
                                    start=False, stop=True,
                                    skip_group_check=True,
                                )
                                def do_y3():
                                    ys = ystage.tile(
                                        [NY, G * BC], f32,
                                        name=f"ys_{g}", tag="ys")
                                    nc.vector.tensor_copy(ys, yp)
                                    nc.gpsimd.dma_start(
                                        out=y_d[:, s_lo * BC:(s_lo + G) * BC],
                                        in_=ys)
                                pending.append(do_y3)
                            pending.append(do_y2)
                        return do_y1
                    pending.append(make_y())

            while pending:
                pending.pop(0)()

            # x_final = V_T rows 0:64 (still eps-scaled; host rescales)
            nc.gpsimd.dma_start(
                out=xf_d[:, :], in_=v_tiles[ngroups - 1][0:NX, 0, :]
            )

    nc.finalize()
    return nc


def _host_prep(x_pred, Y, lambdas, A, B1, B2, C1, D11, D12, C2, D21):
    """Precompute folded weights + per-core packed inputs (all float32)."""
    f = np.float32
    Y_inv = np.linalg.inv(Y.astype(np.float64))
    t_inv = 1.0 / lambdas.astype(np.float64)[:, 0]

    Ab = Y_inv.T @ A.astype(np.float64)   # [nx, nx]
    B1b = Y_inv.T @ B1.astype(np.float64)  # [nx, nu]
    B2b = Y_inv.T @ B2.astype(np.float64)  # [nx, nw]
    C2b = t_inv[:, None] * C2.astype(np.float64)  # [nw, nx]
    D21b = t_inv[:, None] * D21.astype(np.float64)  # [nw, nu]
    C2A = C2b @ Ab    # [nw, nx]
    C2B2 = C2b @ B2b  # [nw, nw]
    C2B1 = C2b @ B1b  # [nw, nu]

    w_step = np.zeros((128, 128), f)
    w_step[0:NX, 0:NX] = Ab.T
    w_step[NX:128, 0:NX] = EPS * B2b.T
    w_step[0:NX, NX:128] = (1.0 / EPS) * C2A.T
    w_step[NX:128, NX:128] = C2B2.T

    wu1 = np.zeros((NU, 128), f)
    wu1[:, 0:NX] = EPS * B1b.T
    wu1[:, NX:128] = C2B1.T
    wu2 = np.zeros((NU, 128), f)
    wu2[:, NX:128] = D21b.T

    w_y = np.zeros((128, NY), f)
    w_y[0:NX, :] = (1.0 / EPS) * C1.T
    w_y[NX:128, :] = D12.T
    w_yu = D11.T.astype(f).copy()

    # feature-major U per core: [nu, T*Bc], col = t*Bc + b
    u_cores = []
    for c in range(NCORES):
        xs = x_pred[c * BC:(c + 1) * BC, :T].astype(f)  # [Bc, T, nu]
        u_cores.append(
            np.ascontiguousarray(xs.transpose(2, 1, 0).reshape(NU, T * BC)))
    return u_cores, dict(w_step=w_step, wu1=wu1, wu2=wu2, w_y=w_y, w_yu=w_yu)


def kernel(x_pred, Y, lambdas, A, B1, B2, C1, D11, D12, C2, D21):
    from concourse.bass_utils import run_bass_kernel_spmd

    u_cores, weights = _host_prep(
        x_pred, Y, lambdas, A, B1, B2, C1, D11, D12, C2, D21)

    if "nc" not in _cached:
        _cached["nc"] = _build_bass()
    nc = _cached["nc"]

    in_maps = [dict(u_feat=u_cores[c], **weights) for c in range(NCORES)]
    res = run_bass_kernel_spmd(nc, in_maps, core_ids=list(range(NCORES)))

    y = np.empty((B, T, NY), np.float32)
    x_final = np.empty((B, NX), np.float32)
    for c in range(NCORES):
        yc = res.results[c]["y_out"].reshape(NY, T, BC)
        y[c * BC:(c + 1) * BC] = yc.transpose(2, 1, 0)
        x_final[c * BC:(c + 1) * BC] = res.results[c]["x_final"].T * (1.0 / EPS)
    return y, x_final
